# revision 26
# baseline (speedup 1.0000x reference)
"""BPNet GNN message-passing kernel for 8 Trainium2 NeuronCores.

Strategy (forced by this image: no extended-GPSIMD ucode, no indirect DMA —
both crash the device, verified experimentally; only static DMA + PE + DVE/ACT
work):
  - Node-sharded output: core c owns nodes [512c, 512c+512). Every (edge,slot)
    pair is routed (host-side marshaling) to the owner core of its target node.
  - Host packs, per core, a position stream of pairs grouped by
    (node-chunk q in [0,4), slot i in [0,3)) -> 12 groups, each padded to whole
    128-position tiles. All per-pair data is baked into dense device tensors:
      xab  [128, L]  bf16 : one-hot-placed gathered node features + type
                            indicator for the pair's two COMPANION slots
                            (K-dim one-hot folds the per-edge weight selection
                            and bias into one fixed matmul).
      msel [128,T*64] bf16 : per-pair output-type selection mask, (d,tau)
                            column order (col 4d+tau)
      oneh [128,T*128]bf16 : per-tile one-hot incidence (lane -> node row)
      cmat/bmat           : per-(slot,type)-group incidence counts + ho_bias
                            rows; bias enters via 4 tiny matmuls.
  - Device pipeline, chunk ch = 512 pair-columns = 4 tiles:
      PE:   pa = w1a^T @ xab[ch], pb = w1b^T @ xab[ch]          (psum)
      ACT:  rb = relu(pb)                                        (sbuf bf16)
      DVE:  fact[ch] = max(pa,0) * rb    (fused scalar_tensor_tensor)
      PE:   pb2[k] = fact_t^T @ w2[:, i_t, :]  (4 tiles -> one psum block)
      POOL: sel = pb2 * msel[ch]                                 (sbuf bf16)
      PE:   pn[:, q, :, :] += oneh_t^T @ sel_t   (psum wide accumulate over
            (d,tau); type-select deferred to a single final tau-reduce)
      PE:   pn[:, q, :, :] += cmat_q^T @ bmat    (bias via group counts)
      DVE:  oc = reduce_tau(pn)  -> [128, 64] f32 -> DMA out
    Stage-2/segsum emission lags stage-1 by 2 chunks so the PE never stalls
    (p-state ramp: PE doubles clock only after ~3us continuous busy).
  - Output per core: [128, 64] f32 = nodes [row, 16q+d]; unshard = pure
    reshape/concat on host.
"""

import numpy as np
import ml_dtypes

N, E, ORDER, D, RANK = 4096, 16384, 3, 13, 128
NP_ = ORDER + 1  # 4 types
NCORES = 8
NODES_PER_CORE = N // NCORES  # 512

bf16 = ml_dtypes.bfloat16

_COMPILED = {}  # Tg -> nc


def _build_program(Tg: int):
    import concourse.bacc as bacc
    import concourse.tile as tile
    from concourse import mybir

    T = 12 * Tg
    L = 128 * T
    NCH = 3 * Tg          # number of 512-col chunks
    CH3 = Tg              # chunks per xab third
    NO = 6                # oneh DMA pieces
    OT = T // NO          # tiles per oneh piece (2*Tg)
    MT = 4 * Tg           # tiles per msel piece

    nc = bacc.Bacc("TRN2", target_bir_lowering=False, debug=False,
                   num_devices=NCORES, enable_partition_id=False)
    BF, F32 = mybir.dt.bfloat16, mybir.dt.float32
    Relu = mybir.ActivationFunctionType.Relu
    Copy = mybir.ActivationFunctionType.Copy
    mx, mult, add = (mybir.AluOpType.max, mybir.AluOpType.mult,
                     mybir.AluOpType.add)

    # xab piece sizes in 512-col chunks: small head so compute starts early
    XP = [1, Tg - 1, Tg, Tg]
    XS = [0, 1, Tg, 2 * Tg]  # piece start chunk

    wpack = nc.dram_tensor("wpack", [128, 1024], BF, kind="ExternalInput").ap()
    xabs = [nc.dram_tensor(f"xab{j}", [128, 512 * XP[j]], BF,
                           kind="ExternalInput").ap() for j in range(4)]
    F8 = mybir.dt.float8e4
    msels = [nc.dram_tensor(f"msel{j}", [128, MT * 64], F8,
                            kind="ExternalInput").ap() for j in range(3)]
    onehs = [nc.dram_tensor(f"oneh{j}", [128, OT * 128], F8,
                            kind="ExternalInput").ap() for j in range(NO)]
    out = nc.dram_tensor("out", [128, 64], F32, kind="ExternalOutput").ap()

    with tile.TileContext(nc) as tc:
        with tc.tile_pool(name="inp", bufs=1) as inp, \
             tc.tile_pool(name="work", bufs=1) as work, \
             tc.tile_pool(name="rbp", bufs=3) as rbp, \
             tc.tile_pool(name="selp", bufs=10) as selp, \
             tc.tile_pool(name="ps1", bufs=4, space="PSUM") as ps1, \
             tc.tile_pool(name="ps2", bufs=2, space="PSUM") as ps2, \
             tc.tile_pool(name="psn", bufs=1, space="PSUM") as psn, \
             tc.tile_pool(name="psw", bufs=1, space="PSUM") as psw:
            wpack_sb = inp.tile([128, 1024], BF, tag="wpack")
            xab_sbs = [inp.tile([128, 512 * XP[j]], BF, tag=f"xab{j}",
                                name=f"xab{j}_sb") for j in range(4)]
            msel_sbs = [inp.tile([128, MT, 64], F8, tag=f"msel{j}",
                                 name=f"msel{j}_sb") for j in range(3)]
            oneh_sbs = [inp.tile([128, OT * 128], F8, tag=f"oneh{j}",
                                 name=f"oneh{j}_sb") for j in range(NO)]

            # DMA issue order == expected consumption order, all on sync so
            # per-queue FIFO keeps bandwidth priority aligned with need time.
            nc.sync.dma_start(wpack_sb[:], wpack[:])
            nc.sync.dma_start(xab_sbs[0][:], xabs[0][:])
            nc.sync.dma_start(xab_sbs[1][:], xabs[1][:])
            nc.sync.dma_start(oneh_sbs[0][:], onehs[0][:])
            nc.sync.dma_start(msel_sbs[0][:, :, :], msels[0][:])
            nc.sync.dma_start(oneh_sbs[1][:], onehs[1][:])
            nc.sync.dma_start(xab_sbs[2][:], xabs[2][:])
            nc.sync.dma_start(oneh_sbs[2][:], onehs[2][:])
            nc.sync.dma_start(msel_sbs[1][:, :, :], msels[1][:])
            nc.sync.dma_start(oneh_sbs[3][:], onehs[3][:])
            nc.sync.dma_start(xab_sbs[3][:], xabs[3][:])
            nc.sync.dma_start(oneh_sbs[4][:], onehs[4][:])
            nc.sync.dma_start(msel_sbs[2][:, :, :], msels[2][:])
            nc.sync.dma_start(oneh_sbs[5][:], onehs[5][:])

            w1a = wpack_sb[:, 0:128]
            w1b = wpack_sb[:, 128:256]
            # w2 columns in (d, tau) order: col 4d+tau
            w2s = [wpack_sb[:, 256 + 64 * i:256 + 64 * (i + 1)]
                   for i in range(3)]
            cmats = [wpack_sb[0:12, 448 + 128 * q:448 + 128 * (q + 1)]
                     for q in range(4)]
            bmat = wpack_sb[0:12, 960:1024]

            fact = work.tile([128, L], BF, tag="fact")
            oc = work.tile([128, 4, 16], F32, tag="oc")
            # node psum split across 2 banks (q0,q1 | q2,q3) so the per-half
            # tau-reduce never blocks later segsum matmuls (no WAR on PE)
            pns = [psn.tile([128, 2, 16, 4], F32, tag="pn", name=f"pn{h}")
                   for h in range(2)]

            # PE p-state warmup: the clock only reaches 2.4GHz after ~3us
            # of continuous busy. Run throwaway matmuls on wpack while the
            # xab0 DMA is still in flight so real work starts at full clock.
            warm = psw.tile([128, 512], F32, tag="warm")
            for w in range(2):
                nc.tensor.matmul(warm[:], w1a, wpack_sb[:, 0:512],
                                 start=True, stop=True)

            def stage1(ch):
                pa = ps1.tile([128, 512], F32, tag="p1")
                pb = ps1.tile([128, 512], F32, tag="p1")
                xp = max(j for j in range(4) if XS[j] <= ch)
                lsl = slice(512 * (ch - XS[xp]), 512 * (ch - XS[xp] + 1))
                xsb = xab_sbs[xp]
                nc.tensor.matmul(pa[:], w1a, xsb[:, lsl], start=True,
                                 stop=True)
                nc.tensor.matmul(pb[:], w1b, xsb[:, lsl], start=True,
                                 stop=True)
                rb = rbp.tile([128, 512], BF, tag="rb")
                nc.scalar.activation(rb[:], pb[:], Relu)
                sl = slice(512 * ch, 512 * (ch + 1))
                nc.vector.scalar_tensor_tensor(fact[:, sl], pa[:], 0.0,
                                               rb[:], op0=mx, op1=mult)

            sels = {}

            def stage2a(ch):
                t0 = 4 * ch
                pb2 = ps2.tile([128, 4, 64], F32, tag="p2")
                for k in range(4):
                    t = t0 + k
                    i_slot = (t // Tg) % 3
                    nc.tensor.matmul(pb2[:, k, :],
                                     fact[:, 128 * t:128 * (t + 1)],
                                     w2s[i_slot], start=True, stop=True)
                # PSUM -> SBUF bf16 copy (Pool cannot touch PSUM); alternate
                # ACT/DVE to balance engine load.
                pc = selp.tile([128, 4, 64], BF, tag="pc")
                if ch % 2 == 0:
                    nc.scalar.activation(pc[:, :, :], pb2[:, :, :], Copy)
                else:
                    nc.vector.tensor_copy(pc[:, :, :], pb2[:, :, :])
                sel = selp.tile([128, 4, 64], BF, tag="sel")
                # Pool handles the select mult mid-phase; in the drain (no
                # more stage-1 work) alternate onto DVE so the last blocks
                # pipeline across two engines instead of serializing on Pool.
                eng = nc.vector if (ch >= NCH - 4 and ch % 2 == 1) else                     nc.gpsimd
                eng.tensor_tensor(
                    sel[:, :, :], pc[:, :, :],
                    msel_sbs[t0 // MT][:, t0 % MT:t0 % MT + 4, :], mult)
                sels[ch] = sel

            def stage2b(ch):
                t0 = 4 * ch
                sel = sels.pop(ch)
                for k in range(4):
                    t = t0 + k
                    q = t // (3 * Tg)
                    h, hq = q // 2, q % 2
                    pn = pns[h]
                    osb = oneh_sbs[t // OT]
                    # start=True only on the first write to each pn bank: it
                    # marks the whole 2KB zero-region pending-zero; each later
                    # region's first touch then auto-zeroes (flag persists).
                    nc.tensor.matmul(
                        pn[:, hq, :, :],
                        osb[:, 128 * (t % OT):128 * (t % OT + 1)],
                        sel[:, k, :],
                        start=(t % (6 * Tg) == 0), stop=False,
                        skip_group_check=True)
                    if t % (6 * Tg) == 0:
                        # bias for this bank's two q windows: first touch of
                        # each window overwrites (pending-zero), later segsums
                        # accumulate on top
                        for q2 in (2 * h, 2 * h + 1):
                            nc.tensor.matmul(pn[:, q2 % 2, :, :], cmats[q2],
                                             bmat, start=False, stop=False,
                                             skip_group_check=True)
                    if (t + 1) % (6 * Tg) == 0:
                        # bank complete: tau-reduce folds the per-pair output-
                        # type selection; stream this half's result out now
                        nc.vector.tensor_reduce(oc[:, 2 * h:2 * h + 2, :],
                                                pn[:, :, :, :],
                                                axis=mybir.AxisListType.X,
                                                op=add)
                        nc.sync.dma_start(out[:, 32 * h:32 * (h + 1)],
                                          oc[:, 2 * h:2 * h + 2, :])

            # stage2a (pb2 matmuls + copy + mult) lags stage1 by 2 chunks;
            # stage2b (segsum matmuls) lags one more, so the PE never sits
            # behind the just-emitted copy->mult chain of the same block.
            for ch in range(NCH + 5):
                if ch < NCH:
                    stage1(ch)
                if 2 <= ch < NCH + 2:
                    stage2a(ch - 2)
                if ch >= 5:
                    stage2b(ch - 5)

    nc.compile()
    return nc


def _prep_inputs(nodes, bp_params, bp_bias, ho_params, ho_bias, edges,
                 edge_types):
    nodes = np.asarray(nodes, np.float32)
    bp_params = np.asarray(bp_params, np.float32)
    bp_bias = np.asarray(bp_bias, np.float32)
    ho_params = np.asarray(ho_params, np.float32)
    ho_bias = np.asarray(ho_bias, np.float32)
    edges = np.asarray(edges, np.int64)
    edge_types = np.asarray(edge_types, np.int64)

    nodes_b = nodes.astype(bf16)

    # group pairs: (core, q, i) ; pair list per group
    tgt = edges  # [E, 3]
    owner = tgt // NODES_PER_CORE
    q = (tgt % NODES_PER_CORE) // 128
    r = tgt % 128

    group_lists = {}
    maxcount = 1
    for c in range(NCORES):
        for i in range(ORDER):
            sel_c = owner[:, i] == c
            for qq in range(4):
                es = np.nonzero(sel_c & (q[:, i] == qq))[0]
                group_lists[(c, qq, i)] = es
                maxcount = max(maxcount, len(es))
    Tg = (maxcount + 127) // 128
    T = 12 * Tg
    L = 128 * T
    NO = 6
    OT = T // NO
    MT = 4 * Tg

    # packed weight tables (shared across cores)
    wpack = np.zeros((128, 1024), np.float32)
    for p in range(NP_):
        wpack[13 * p:13 * p + 13, 0:128] = bp_params[p]
        wpack[52 + p, 0:128] = bp_bias[p, 0, :]
        wpack[64 + 13 * p:64 + 13 * p + 13, 128:256] = bp_params[p]
        wpack[116 + p, 128:256] = bp_bias[p, 0, :]
    # w2 columns in (d, tau) order
    for i in range(ORDER):
        for p in range(NP_):
            for dd in range(D):
                wpack[:, 256 + 64 * i + 4 * dd + p] = ho_params[i, p, :, dd]
    # bmat: bias rows per group g=4i+p, tau=0 slot (col 4d)
    for i in range(ORDER):
        for p in range(NP_):
            for dd in range(D):
                wpack[4 * i + p, 960 + 4 * dd] = ho_bias[i, p, 0, dd]

    in_maps = []
    for c in range(NCORES):
        xab = np.zeros((128, L), np.float32)
        msel = np.zeros((128, T * 64), np.float32)
        oneh = np.zeros((128, T * 128), np.float32)
        wp = wpack.copy()
        for qq in range(4):
            for i in range(ORDER):
                es = group_lists[(c, qq, i)]
                g = qq * 3 + i
                k = np.arange(len(es))
                x = 128 * g * Tg + k
                t_arr = x // 128
                lane = x % 128
                a, b = (i + 1) % 3, (i + 2) % 3
                ta_t = edge_types[es, a]
                tb_t = edge_types[es, b]
                fa = nodes_b[edges[es, a]].astype(np.float32)  # [m, 13]
                fb = nodes_b[edges[es, b]].astype(np.float32)
                for dd in range(D):
                    xab[13 * ta_t + dd, x] = fa[:, dd]
                    xab[64 + 13 * tb_t + dd, x] = fb[:, dd]
                xab[52 + ta_t, x] = 1.0
                xab[116 + tb_t, x] = 1.0
                p_e = edge_types[es, i]
                for dd in range(D):
                    msel[lane, 64 * t_arr + 4 * dd + p_e] = 1.0
                oneh[lane, 128 * t_arr + r[es, i]] = 1.0
                # bias count matrix: cmat[g=4i+p, q, node_row]
                cnt = np.bincount(r[es, i] + 128 * p_e, minlength=128 * NP_)
                for p in range(NP_):
                    wp[4 * i + p, 448 + 128 * qq:448 + 128 * (qq + 1)] += \
                        cnt[128 * p:128 * (p + 1)]
        import ml_dtypes as _mld
        f8 = _mld.float8_e4m3
        xab_b = xab.astype(bf16)
        oneh_b = oneh.astype(f8)
        msel_b = msel.astype(f8)
        m = {"wpack": wp.astype(bf16)}
        xs = [0, 1, Tg, 2 * Tg, 3 * Tg]  # xab piece boundaries in chunks
        for j in range(4):
            m[f"xab{j}"] = xab_b[:, 512 * xs[j]:512 * xs[j + 1]]
        for j in range(3):
            m[f"msel{j}"] = msel_b[:, j * MT * 64:(j + 1) * MT * 64]
        for j in range(NO):
            m[f"oneh{j}"] = oneh_b[:, j * OT * 128:(j + 1) * OT * 128]
        in_maps.append(m)
    return in_maps, Tg


def kernel(nodes, bp_params, bp_bias, ho_params, ho_bias, edges, edge_types,
           atoms=None, atom_edges=None, _run_kwargs=None):
    from concourse.bass_utils import run_bass_kernel_spmd

    in_maps, Tg = _prep_inputs(nodes, bp_params, bp_bias, ho_params, ho_bias,
                               edges, edge_types)
    if Tg not in _COMPILED:
        _COMPILED[Tg] = _build_program(Tg)
    nc = _COMPILED[Tg]

    res = run_bass_kernel_spmd(nc, in_maps, core_ids=list(range(NCORES)),
                               **(_run_kwargs or {}))
    full = np.zeros((N, D), np.float32)
    for c in range(NCORES):
        oc = res.results[c]["out"]  # [128, 64] = [128, 4q, 16d]
        for qq in range(4):
            full[NODES_PER_CORE * c + 128 * qq:
                 NODES_PER_CORE * c + 128 * (qq + 1), :] = oc[:, 16 * qq:
                                                             16 * qq + 13]
    kernel._last_result = res
    return full


# revision 27
# speedup vs baseline: 1.0195x; 1.0195x over previous
"""BPNet GNN message-passing kernel for 8 Trainium2 NeuronCores.

Strategy (forced by this image: no extended-GPSIMD ucode, no indirect DMA —
both crash the device, verified experimentally; only static DMA + PE + DVE/ACT
work):
  - Node-sharded output: core c owns nodes [512c, 512c+512). Every (edge,slot)
    pair is routed (host-side marshaling) to the owner core of its target node.
  - Host packs, per core, a position stream of pairs grouped by
    (node-chunk q in [0,4), slot i in [0,3)) -> 12 groups, each padded to whole
    128-position tiles. All per-pair data is baked into dense device tensors:
      xab  [128, L]  bf16 : one-hot-placed gathered node features + type
                            indicator for the pair's two COMPANION slots
                            (K-dim one-hot folds the per-edge weight selection
                            and bias into one fixed matmul).
      msel [128,T*64] bf16 : per-pair output-type selection mask, (d,tau)
                            column order (col 4d+tau)
      oneh [128,T*128]bf16 : per-tile one-hot incidence (lane -> node row)
      cmat/bmat           : per-(slot,type)-group incidence counts + ho_bias
                            rows; bias enters via 4 tiny matmuls.
  - Device pipeline, chunk ch = 512 pair-columns = 4 tiles:
      PE:   pa = w1a^T @ xab[ch], pb = w1b^T @ xab[ch]          (psum)
      ACT:  rb = relu(pb)                                        (sbuf bf16)
      DVE:  fact[ch] = max(pa,0) * rb    (fused scalar_tensor_tensor)
      PE:   pb2[k] = fact_t^T @ w2[:, i_t, :]  (4 tiles -> one psum block)
      POOL: sel = pb2 * msel[ch]                                 (sbuf bf16)
      PE:   pn[:, q, :, :] += oneh_t^T @ sel_t   (psum wide accumulate over
            (d,tau); type-select deferred to a single final tau-reduce)
      PE:   pn[:, q, :, :] += cmat_q^T @ bmat    (bias via group counts)
      DVE:  oc = reduce_tau(pn)  -> [128, 64] f32 -> DMA out
    Stage-2/segsum emission lags stage-1 by 2 chunks so the PE never stalls
    (p-state ramp: PE doubles clock only after ~3us continuous busy).
  - Output per core: [128, 64] f32 = nodes [row, 16q+d]; unshard = pure
    reshape/concat on host.
"""

import numpy as np
import ml_dtypes

N, E, ORDER, D, RANK = 4096, 16384, 3, 13, 128
NP_ = ORDER + 1  # 4 types
NCORES = 8
NODES_PER_CORE = N // NCORES  # 512

bf16 = ml_dtypes.bfloat16

_COMPILED = {}  # Tg -> nc


def _build_program(Tg: int):
    import concourse.bacc as bacc
    import concourse.tile as tile
    from concourse import mybir

    T = 12 * Tg
    L = 128 * T
    NCH = 3 * Tg          # number of 512-col chunks
    CH3 = Tg              # chunks per xab third
    NO = 6                # oneh DMA pieces
    OT = T // NO          # tiles per oneh piece (2*Tg)
    MT = 4 * Tg           # tiles per msel piece

    nc = bacc.Bacc("TRN2", target_bir_lowering=False, debug=False,
                   num_devices=NCORES, enable_partition_id=False)
    BF, F32 = mybir.dt.bfloat16, mybir.dt.float32
    Relu = mybir.ActivationFunctionType.Relu
    Copy = mybir.ActivationFunctionType.Copy
    mx, mult, add = (mybir.AluOpType.max, mybir.AluOpType.mult,
                     mybir.AluOpType.add)

    # xab piece sizes in 512-col chunks: small head so compute starts early
    XP = [1, Tg - 1, Tg, Tg]
    XS = [0, 1, Tg, 2 * Tg]  # piece start chunk

    wpack = nc.dram_tensor("wpack", [128, 1024], BF, kind="ExternalInput").ap()
    xabs = [nc.dram_tensor(f"xab{j}", [128, 512 * XP[j]], BF,
                           kind="ExternalInput").ap() for j in range(4)]
    F8 = mybir.dt.float8e4
    msels = [nc.dram_tensor(f"msel{j}", [128, MT * 64], F8,
                            kind="ExternalInput").ap() for j in range(3)]
    onehs = [nc.dram_tensor(f"oneh{j}", [128, OT * 128], F8,
                            kind="ExternalInput").ap() for j in range(NO)]
    out = nc.dram_tensor("out", [128, 64], F32, kind="ExternalOutput").ap()

    with tile.TileContext(nc) as tc:
        with tc.tile_pool(name="inp", bufs=1) as inp, \
             tc.tile_pool(name="work", bufs=1) as work, \
             tc.tile_pool(name="rbp", bufs=3) as rbp, \
             tc.tile_pool(name="selp", bufs=10) as selp, \
             tc.tile_pool(name="ps1", bufs=4, space="PSUM") as ps1, \
             tc.tile_pool(name="ps2", bufs=3, space="PSUM") as ps2, \
             tc.tile_pool(name="psn", bufs=1, space="PSUM") as psn:
            wpack_sb = inp.tile([128, 1024], BF, tag="wpack")
            xab_sbs = [inp.tile([128, 512 * XP[j]], BF, tag=f"xab{j}",
                                name=f"xab{j}_sb") for j in range(4)]
            msel_sbs = [inp.tile([128, MT, 64], F8, tag=f"msel{j}",
                                 name=f"msel{j}_sb") for j in range(3)]
            oneh_sbs = [inp.tile([128, OT * 128], F8, tag=f"oneh{j}",
                                 name=f"oneh{j}_sb") for j in range(NO)]

            # DMA issue order == expected consumption order, all on sync so
            # per-queue FIFO keeps bandwidth priority aligned with need time.
            nc.sync.dma_start(wpack_sb[:], wpack[:])
            nc.sync.dma_start(xab_sbs[0][:], xabs[0][:])
            nc.sync.dma_start(xab_sbs[1][:], xabs[1][:])
            nc.sync.dma_start(oneh_sbs[0][:], onehs[0][:])
            nc.sync.dma_start(msel_sbs[0][:, :, :], msels[0][:])
            nc.sync.dma_start(oneh_sbs[1][:], onehs[1][:])
            nc.sync.dma_start(xab_sbs[2][:], xabs[2][:])
            nc.sync.dma_start(oneh_sbs[2][:], onehs[2][:])
            nc.sync.dma_start(msel_sbs[1][:, :, :], msels[1][:])
            nc.sync.dma_start(oneh_sbs[3][:], onehs[3][:])
            nc.sync.dma_start(xab_sbs[3][:], xabs[3][:])
            nc.sync.dma_start(oneh_sbs[4][:], onehs[4][:])
            nc.sync.dma_start(msel_sbs[2][:, :, :], msels[2][:])
            nc.sync.dma_start(oneh_sbs[5][:], onehs[5][:])

            w1a = wpack_sb[:, 0:128]
            w1b = wpack_sb[:, 128:256]
            # w2 columns in (d, tau) order: col 4d+tau
            w2s = [wpack_sb[:, 256 + 64 * i:256 + 64 * (i + 1)]
                   for i in range(3)]
            cmats = [wpack_sb[0:12, 448 + 128 * q:448 + 128 * (q + 1)]
                     for q in range(4)]
            bmat = wpack_sb[0:12, 960:1024]

            fact = work.tile([128, L], BF, tag="fact")
            oc = work.tile([128, 4, 16], F32, tag="oc")
            # node psum split across 2 banks (q0,q1 | q2,q3) so the per-half
            # tau-reduce never blocks later segsum matmuls (no WAR on PE)
            pns = [psn.tile([128, 2, 16, 4], F32, tag="pn", name=f"pn{h}")
                   for h in range(2)]

            def stage1(ch):
                pa = ps1.tile([128, 512], F32, tag="p1")
                pb = ps1.tile([128, 512], F32, tag="p1")
                xp = max(j for j in range(4) if XS[j] <= ch)
                lsl = slice(512 * (ch - XS[xp]), 512 * (ch - XS[xp] + 1))
                xsb = xab_sbs[xp]
                nc.tensor.matmul(pa[:], w1a, xsb[:, lsl], start=True,
                                 stop=True)
                nc.tensor.matmul(pb[:], w1b, xsb[:, lsl], start=True,
                                 stop=True)
                rb = rbp.tile([128, 512], BF, tag="rb")
                nc.scalar.activation(rb[:], pb[:], Relu)
                sl = slice(512 * ch, 512 * (ch + 1))
                nc.vector.scalar_tensor_tensor(fact[:, sl], pa[:], 0.0,
                                               rb[:], op0=mx, op1=mult)

            sels = {}

            def stage2a(ch):
                t0 = 4 * ch
                pb2 = ps2.tile([128, 4, 64], F32, tag="p2")
                for k in range(4):
                    t = t0 + k
                    i_slot = (t // Tg) % 3
                    nc.tensor.matmul(pb2[:, k, :],
                                     fact[:, 128 * t:128 * (t + 1)],
                                     w2s[i_slot], start=True, stop=True)
                # PSUM -> SBUF bf16 copy (Pool cannot touch PSUM); alternate
                # ACT/DVE to balance engine load.
                pc = selp.tile([128, 4, 64], BF, tag="pc")
                if ch % 2 == 0:
                    nc.scalar.activation(pc[:, :, :], pb2[:, :, :], Copy)
                else:
                    nc.vector.tensor_copy(pc[:, :, :], pb2[:, :, :])
                sel = selp.tile([128, 4, 64], BF, tag="sel")
                # Pool handles the select mult mid-phase; in the drain (no
                # more stage-1 work) alternate onto DVE so the last blocks
                # pipeline across two engines instead of serializing on Pool.
                eng = nc.vector if (ch >= NCH - 4 and ch % 2 == 1) else                     nc.gpsimd
                eng.tensor_tensor(
                    sel[:, :, :], pc[:, :, :],
                    msel_sbs[t0 // MT][:, t0 % MT:t0 % MT + 4, :], mult)
                sels[ch] = sel

            def stage2b(ch):
                t0 = 4 * ch
                sel = sels.pop(ch)
                for k in range(4):
                    t = t0 + k
                    q = t // (3 * Tg)
                    h, hq = q // 2, q % 2
                    pn = pns[h]
                    osb = oneh_sbs[t // OT]
                    # start=True only on the first write to each pn bank: it
                    # marks the whole 2KB zero-region pending-zero; each later
                    # region's first touch then auto-zeroes (flag persists).
                    nc.tensor.matmul(
                        pn[:, hq, :, :],
                        osb[:, 128 * (t % OT):128 * (t % OT + 1)],
                        sel[:, k, :],
                        start=(t % (6 * Tg) == 0), stop=False,
                        skip_group_check=True)
                    if t % (6 * Tg) == 0:
                        # bias for this bank's two q windows: first touch of
                        # each window overwrites (pending-zero), later segsums
                        # accumulate on top
                        for q2 in (2 * h, 2 * h + 1):
                            nc.tensor.matmul(pn[:, q2 % 2, :, :], cmats[q2],
                                             bmat, start=False, stop=False,
                                             skip_group_check=True)
                    if (t + 1) % (6 * Tg) == 0:
                        # bank complete: tau-reduce folds the per-pair output-
                        # type selection; stream this half's result out now
                        nc.vector.tensor_reduce(oc[:, 2 * h:2 * h + 2, :],
                                                pn[:, :, :, :],
                                                axis=mybir.AxisListType.X,
                                                op=add)
                        nc.sync.dma_start(out[:, 32 * h:32 * (h + 1)],
                                          oc[:, 2 * h:2 * h + 2, :])

            # stage2a (pb2 matmuls + copy + mult) lags stage1 by 2 chunks;
            # stage2b (segsum matmuls) lags one more, so the PE never sits
            # behind the just-emitted copy->mult chain of the same block.
            for ch in range(NCH + 5):
                if ch < NCH:
                    stage1(ch)
                if 2 <= ch < NCH + 2:
                    stage2a(ch - 2)
                if ch >= 5:
                    stage2b(ch - 5)

    nc.compile()
    return nc


def _prep_inputs(nodes, bp_params, bp_bias, ho_params, ho_bias, edges,
                 edge_types):
    nodes = np.asarray(nodes, np.float32)
    bp_params = np.asarray(bp_params, np.float32)
    bp_bias = np.asarray(bp_bias, np.float32)
    ho_params = np.asarray(ho_params, np.float32)
    ho_bias = np.asarray(ho_bias, np.float32)
    edges = np.asarray(edges, np.int64)
    edge_types = np.asarray(edge_types, np.int64)

    nodes_b = nodes.astype(bf16)

    # group pairs: (core, q, i) ; pair list per group
    tgt = edges  # [E, 3]
    owner = tgt // NODES_PER_CORE
    q = (tgt % NODES_PER_CORE) // 128
    r = tgt % 128

    group_lists = {}
    maxcount = 1
    for c in range(NCORES):
        for i in range(ORDER):
            sel_c = owner[:, i] == c
            for qq in range(4):
                es = np.nonzero(sel_c & (q[:, i] == qq))[0]
                group_lists[(c, qq, i)] = es
                maxcount = max(maxcount, len(es))
    Tg = (maxcount + 127) // 128
    T = 12 * Tg
    L = 128 * T
    NO = 6
    OT = T // NO
    MT = 4 * Tg

    # packed weight tables (shared across cores)
    wpack = np.zeros((128, 1024), np.float32)
    for p in range(NP_):
        wpack[13 * p:13 * p + 13, 0:128] = bp_params[p]
        wpack[52 + p, 0:128] = bp_bias[p, 0, :]
        wpack[64 + 13 * p:64 + 13 * p + 13, 128:256] = bp_params[p]
        wpack[116 + p, 128:256] = bp_bias[p, 0, :]
    # w2 columns in (d, tau) order
    for i in range(ORDER):
        for p in range(NP_):
            for dd in range(D):
                wpack[:, 256 + 64 * i + 4 * dd + p] = ho_params[i, p, :, dd]
    # bmat: bias rows per group g=4i+p, tau=0 slot (col 4d)
    for i in range(ORDER):
        for p in range(NP_):
            for dd in range(D):
                wpack[4 * i + p, 960 + 4 * dd] = ho_bias[i, p, 0, dd]

    in_maps = []
    for c in range(NCORES):
        xab = np.zeros((128, L), np.float32)
        msel = np.zeros((128, T * 64), np.float32)
        oneh = np.zeros((128, T * 128), np.float32)
        wp = wpack.copy()
        for qq in range(4):
            for i in range(ORDER):
                es = group_lists[(c, qq, i)]
                g = qq * 3 + i
                k = np.arange(len(es))
                x = 128 * g * Tg + k
                t_arr = x // 128
                lane = x % 128
                a, b = (i + 1) % 3, (i + 2) % 3
                ta_t = edge_types[es, a]
                tb_t = edge_types[es, b]
                fa = nodes_b[edges[es, a]].astype(np.float32)  # [m, 13]
                fb = nodes_b[edges[es, b]].astype(np.float32)
                for dd in range(D):
                    xab[13 * ta_t + dd, x] = fa[:, dd]
                    xab[64 + 13 * tb_t + dd, x] = fb[:, dd]
                xab[52 + ta_t, x] = 1.0
                xab[116 + tb_t, x] = 1.0
                p_e = edge_types[es, i]
                for dd in range(D):
                    msel[lane, 64 * t_arr + 4 * dd + p_e] = 1.0
                oneh[lane, 128 * t_arr + r[es, i]] = 1.0
                # bias count matrix: cmat[g=4i+p, q, node_row]
                cnt = np.bincount(r[es, i] + 128 * p_e, minlength=128 * NP_)
                for p in range(NP_):
                    wp[4 * i + p, 448 + 128 * qq:448 + 128 * (qq + 1)] += \
                        cnt[128 * p:128 * (p + 1)]
        import ml_dtypes as _mld
        f8 = _mld.float8_e4m3
        xab_b = xab.astype(bf16)
        oneh_b = oneh.astype(f8)
        msel_b = msel.astype(f8)
        m = {"wpack": wp.astype(bf16)}
        xs = [0, 1, Tg, 2 * Tg, 3 * Tg]  # xab piece boundaries in chunks
        for j in range(4):
            m[f"xab{j}"] = xab_b[:, 512 * xs[j]:512 * xs[j + 1]]
        for j in range(3):
            m[f"msel{j}"] = msel_b[:, j * MT * 64:(j + 1) * MT * 64]
        for j in range(NO):
            m[f"oneh{j}"] = oneh_b[:, j * OT * 128:(j + 1) * OT * 128]
        in_maps.append(m)
    return in_maps, Tg


def kernel(nodes, bp_params, bp_bias, ho_params, ho_bias, edges, edge_types,
           atoms=None, atom_edges=None, _run_kwargs=None):
    from concourse.bass_utils import run_bass_kernel_spmd

    in_maps, Tg = _prep_inputs(nodes, bp_params, bp_bias, ho_params, ho_bias,
                               edges, edge_types)
    if Tg not in _COMPILED:
        _COMPILED[Tg] = _build_program(Tg)
    nc = _COMPILED[Tg]

    res = run_bass_kernel_spmd(nc, in_maps, core_ids=list(range(NCORES)),
                               **(_run_kwargs or {}))
    full = np.zeros((N, D), np.float32)
    for c in range(NCORES):
        oc = res.results[c]["out"]  # [128, 64] = [128, 4q, 16d]
        for qq in range(4):
            full[NODES_PER_CORE * c + 128 * qq:
                 NODES_PER_CORE * c + 128 * (qq + 1), :] = oc[:, 16 * qq:
                                                             16 * qq + 13]
    kernel._last_result = res
    return full


# revision 28
# speedup vs baseline: 1.0798x; 1.0591x over previous
"""BPNet GNN message-passing kernel for 8 Trainium2 NeuronCores.

Strategy (forced by this image: no extended-GPSIMD ucode, no indirect DMA —
both crash the device, verified experimentally; only static DMA + PE + DVE/ACT
work):
  - Node-sharded output: core c owns nodes [512c, 512c+512). Every (edge,slot)
    pair is routed (host-side marshaling) to the owner core of its target node.
  - Host packs, per core, a position stream of pairs grouped by
    (node-chunk q in [0,4), slot i in [0,3)) -> 12 groups, each padded to whole
    128-position tiles. All per-pair data is baked into dense device tensors:
      xab  [128, L]  bf16 : one-hot-placed gathered node features + type
                            indicator for the pair's two COMPANION slots
                            (K-dim one-hot folds the per-edge weight selection
                            and bias into one fixed matmul).
      msel [128,T*64] bf16 : per-pair output-type selection mask, (d,tau)
                            column order (col 4d+tau)
      oneh [128,T*128]bf16 : per-tile one-hot incidence (lane -> node row)
      cmat/bmat           : per-(slot,type)-group incidence counts + ho_bias
                            rows; bias enters via 4 tiny matmuls.
  - Device pipeline, chunk ch = 512 pair-columns = 4 tiles:
      PE:   pa = w1a^T @ xab[ch], pb = w1b^T @ xab[ch]          (psum)
      ACT:  rb = relu(pb)                                        (sbuf bf16)
      DVE:  fact[ch] = max(pa,0) * rb    (fused scalar_tensor_tensor)
      PE:   pb2[k] = fact_t^T @ w2[:, i_t, :]  (4 tiles -> one psum block)
      POOL: sel = pb2 * msel[ch]                                 (sbuf bf16)
      PE:   pn[:, q, :, :] += oneh_t^T @ sel_t   (psum wide accumulate over
            (d,tau); type-select deferred to a single final tau-reduce)
      PE:   pn[:, q, :, :] += cmat_q^T @ bmat    (bias via group counts)
      DVE:  oc = reduce_tau(pn)  -> [128, 64] f32 -> DMA out
    Stage-2/segsum emission lags stage-1 by 2 chunks so the PE never stalls
    (p-state ramp: PE doubles clock only after ~3us continuous busy).
  - Output per core: [128, 64] f32 = nodes [row, 16q+d]; unshard = pure
    reshape/concat on host.
"""

import numpy as np
import ml_dtypes

N, E, ORDER, D, RANK = 4096, 16384, 3, 13, 128
NP_ = ORDER + 1  # 4 types
NCORES = 8
NODES_PER_CORE = N // NCORES  # 512

bf16 = ml_dtypes.bfloat16

_COMPILED = {}  # Tg -> nc


def _build_program(Tg: int):
    import concourse.bacc as bacc
    import concourse.tile as tile
    from concourse import mybir

    T = 12 * Tg
    L = 128 * T
    NCH = 3 * Tg          # number of 512-col chunks
    CH3 = Tg              # chunks per xab third
    NO = 6                # oneh DMA pieces
    OT = T // NO          # tiles per oneh piece (2*Tg)
    MT = 4 * Tg           # tiles per msel piece

    nc = bacc.Bacc("TRN2", target_bir_lowering=False, debug=False,
                   num_devices=NCORES)
    BF, F32 = mybir.dt.bfloat16, mybir.dt.float32
    Relu = mybir.ActivationFunctionType.Relu
    Copy = mybir.ActivationFunctionType.Copy
    mx, mult, add = (mybir.AluOpType.max, mybir.AluOpType.mult,
                     mybir.AluOpType.add)

    # xab piece sizes in 512-col chunks: small head so compute starts early
    XP = [2, Tg - 2, Tg, Tg]
    XS = [0, 2, Tg, 2 * Tg]  # piece start chunk

    wpack = nc.dram_tensor("wpack", [128, 1024], BF, kind="ExternalInput").ap()
    xabs = [nc.dram_tensor(f"xab{j}", [128, 512 * XP[j]], BF,
                           kind="ExternalInput").ap() for j in range(4)]
    F8 = mybir.dt.float8e4
    msels = [nc.dram_tensor(f"msel{j}", [128, MT * 64], F8,
                            kind="ExternalInput").ap() for j in range(3)]
    onehs = [nc.dram_tensor(f"oneh{j}", [128, OT * 128], F8,
                            kind="ExternalInput").ap() for j in range(NO)]
    out = nc.dram_tensor("out", [128, 64], F32, kind="ExternalOutput").ap()

    with tile.TileContext(nc) as tc:
        with tc.tile_pool(name="inp", bufs=1) as inp, \
             tc.tile_pool(name="work", bufs=1) as work, \
             tc.tile_pool(name="rbp", bufs=3) as rbp, \
             tc.tile_pool(name="selp", bufs=10) as selp, \
             tc.tile_pool(name="ps1", bufs=4, space="PSUM") as ps1, \
             tc.tile_pool(name="ps2", bufs=3, space="PSUM") as ps2, \
             tc.tile_pool(name="psn", bufs=1, space="PSUM") as psn:
            wpack_sb = inp.tile([128, 1024], BF, tag="wpack")
            xab_sbs = [inp.tile([128, 512 * XP[j]], BF, tag=f"xab{j}",
                                name=f"xab{j}_sb") for j in range(4)]
            msel_sbs = [inp.tile([128, MT, 64], F8, tag=f"msel{j}",
                                 name=f"msel{j}_sb") for j in range(3)]
            oneh_sbs = [inp.tile([128, OT * 128], F8, tag=f"oneh{j}",
                                 name=f"oneh{j}_sb") for j in range(NO)]

            # DMA issue order == expected consumption order, all on sync so
            # per-queue FIFO keeps bandwidth priority aligned with need time.
            nc.sync.dma_start(wpack_sb[:], wpack[:])
            nc.sync.dma_start(xab_sbs[0][:], xabs[0][:])
            nc.sync.dma_start(xab_sbs[1][:], xabs[1][:])
            nc.sync.dma_start(oneh_sbs[0][:], onehs[0][:])
            nc.sync.dma_start(msel_sbs[0][:, :, :], msels[0][:])
            nc.sync.dma_start(oneh_sbs[1][:], onehs[1][:])
            nc.sync.dma_start(xab_sbs[2][:], xabs[2][:])
            nc.sync.dma_start(oneh_sbs[2][:], onehs[2][:])
            nc.sync.dma_start(msel_sbs[1][:, :, :], msels[1][:])
            nc.sync.dma_start(oneh_sbs[3][:], onehs[3][:])
            nc.sync.dma_start(xab_sbs[3][:], xabs[3][:])
            nc.sync.dma_start(oneh_sbs[4][:], onehs[4][:])
            nc.sync.dma_start(msel_sbs[2][:, :, :], msels[2][:])
            nc.sync.dma_start(oneh_sbs[5][:], onehs[5][:])

            w1a = wpack_sb[:, 0:128]
            w1b = wpack_sb[:, 128:256]
            # w2 columns in (d, tau) order: col 4d+tau
            w2s = [wpack_sb[:, 256 + 64 * i:256 + 64 * (i + 1)]
                   for i in range(3)]
            cmats = [wpack_sb[0:12, 448 + 128 * q:448 + 128 * (q + 1)]
                     for q in range(4)]
            bmat = wpack_sb[0:12, 960:1024]

            fact = work.tile([128, L], BF, tag="fact")
            oc = work.tile([128, 4, 16], F32, tag="oc")
            # node psum split across 2 banks (q0,q1 | q2,q3) so the per-half
            # tau-reduce never blocks later segsum matmuls (no WAR on PE)
            pns = [psn.tile([128, 2, 16, 4], F32, tag="pn", name=f"pn{h}")
                   for h in range(2)]

            def stage1(ch):
                pa = ps1.tile([128, 512], F32, tag="p1")
                pb = ps1.tile([128, 512], F32, tag="p1")
                xp = max(j for j in range(4) if XS[j] <= ch)
                lsl = slice(512 * (ch - XS[xp]), 512 * (ch - XS[xp] + 1))
                xsb = xab_sbs[xp]
                nc.tensor.matmul(pa[:], w1a, xsb[:, lsl], start=True,
                                 stop=True)
                nc.tensor.matmul(pb[:], w1b, xsb[:, lsl], start=True,
                                 stop=True)
                rb = rbp.tile([128, 512], BF, tag="rb")
                nc.scalar.activation(rb[:], pb[:], Relu)
                sl = slice(512 * ch, 512 * (ch + 1))
                nc.vector.scalar_tensor_tensor(fact[:, sl], pa[:], 0.0,
                                               rb[:], op0=mx, op1=mult)

            sels = {}

            def stage2a(ch):
                t0 = 4 * ch
                pb2 = ps2.tile([128, 4, 64], F32, tag="p2")
                for k in range(4):
                    t = t0 + k
                    i_slot = (t // Tg) % 3
                    nc.tensor.matmul(pb2[:, k, :],
                                     fact[:, 128 * t:128 * (t + 1)],
                                     w2s[i_slot], start=True, stop=True)
                # PSUM -> SBUF bf16 copy (Pool cannot touch PSUM); alternate
                # ACT/DVE to balance engine load.
                pc = selp.tile([128, 4, 64], BF, tag="pc")
                if ch % 2 == 0:
                    nc.scalar.activation(pc[:, :, :], pb2[:, :, :], Copy)
                else:
                    nc.vector.tensor_copy(pc[:, :, :], pb2[:, :, :])
                sel = selp.tile([128, 4, 64], BF, tag="sel")
                # Pool handles the select mult mid-phase; in the drain (no
                # more stage-1 work) alternate onto DVE so the last blocks
                # pipeline across two engines instead of serializing on Pool.
                eng = nc.vector if (ch >= NCH - 4 and ch % 2 == 1) else                     nc.gpsimd
                eng.tensor_tensor(
                    sel[:, :, :], pc[:, :, :],
                    msel_sbs[t0 // MT][:, t0 % MT:t0 % MT + 4, :], mult)
                sels[ch] = sel

            def stage2b(ch):
                t0 = 4 * ch
                sel = sels.pop(ch)
                for k in range(4):
                    t = t0 + k
                    q = t // (3 * Tg)
                    h, hq = q // 2, q % 2
                    pn = pns[h]
                    osb = oneh_sbs[t // OT]
                    # start=True only on the first write to each pn bank: it
                    # marks the whole 2KB zero-region pending-zero; each later
                    # region's first touch then auto-zeroes (flag persists).
                    nc.tensor.matmul(
                        pn[:, hq, :, :],
                        osb[:, 128 * (t % OT):128 * (t % OT + 1)],
                        sel[:, k, :],
                        start=(t % (6 * Tg) == 0), stop=False,
                        skip_group_check=True)
                    if t % (6 * Tg) == 0:
                        # bias for this bank's two q windows: first touch of
                        # each window overwrites (pending-zero), later segsums
                        # accumulate on top
                        for q2 in (2 * h, 2 * h + 1):
                            nc.tensor.matmul(pn[:, q2 % 2, :, :], cmats[q2],
                                             bmat, start=False, stop=False,
                                             skip_group_check=True)
                    if (t + 1) % (6 * Tg) == 0:
                        # bank complete: tau-reduce folds the per-pair output-
                        # type selection; stream this half's result out now
                        nc.vector.tensor_reduce(oc[:, 2 * h:2 * h + 2, :],
                                                pn[:, :, :, :],
                                                axis=mybir.AxisListType.X,
                                                op=add)
                        nc.sync.dma_start(out[:, 32 * h:32 * (h + 1)],
                                          oc[:, 2 * h:2 * h + 2, :])

            # stage2a (pb2 matmuls + copy + mult) lags stage1 by 2 chunks;
            # stage2b (segsum matmuls) lags one more, so the PE never sits
            # behind the just-emitted copy->mult chain of the same block.
            for ch in range(NCH + 5):
                if ch < NCH:
                    stage1(ch)
                if 2 <= ch < NCH + 2:
                    stage2a(ch - 2)
                if ch >= 5:
                    stage2b(ch - 5)

    nc.compile()
    return nc


def _prep_inputs(nodes, bp_params, bp_bias, ho_params, ho_bias, edges,
                 edge_types):
    nodes = np.asarray(nodes, np.float32)
    bp_params = np.asarray(bp_params, np.float32)
    bp_bias = np.asarray(bp_bias, np.float32)
    ho_params = np.asarray(ho_params, np.float32)
    ho_bias = np.asarray(ho_bias, np.float32)
    edges = np.asarray(edges, np.int64)
    edge_types = np.asarray(edge_types, np.int64)

    nodes_b = nodes.astype(bf16)

    # group pairs: (core, q, i) ; pair list per group
    tgt = edges  # [E, 3]
    owner = tgt // NODES_PER_CORE
    q = (tgt % NODES_PER_CORE) // 128
    r = tgt % 128

    group_lists = {}
    maxcount = 1
    for c in range(NCORES):
        for i in range(ORDER):
            sel_c = owner[:, i] == c
            for qq in range(4):
                es = np.nonzero(sel_c & (q[:, i] == qq))[0]
                group_lists[(c, qq, i)] = es
                maxcount = max(maxcount, len(es))
    Tg = (maxcount + 127) // 128
    T = 12 * Tg
    L = 128 * T
    NO = 6
    OT = T // NO
    MT = 4 * Tg

    # packed weight tables (shared across cores)
    wpack = np.zeros((128, 1024), np.float32)
    for p in range(NP_):
        wpack[13 * p:13 * p + 13, 0:128] = bp_params[p]
        wpack[52 + p, 0:128] = bp_bias[p, 0, :]
        wpack[64 + 13 * p:64 + 13 * p + 13, 128:256] = bp_params[p]
        wpack[116 + p, 128:256] = bp_bias[p, 0, :]
    # w2 columns in (d, tau) order
    for i in range(ORDER):
        for p in range(NP_):
            for dd in range(D):
                wpack[:, 256 + 64 * i + 4 * dd + p] = ho_params[i, p, :, dd]
    # bmat: bias rows per group g=4i+p, tau=0 slot (col 4d)
    for i in range(ORDER):
        for p in range(NP_):
            for dd in range(D):
                wpack[4 * i + p, 960 + 4 * dd] = ho_bias[i, p, 0, dd]

    in_maps = []
    for c in range(NCORES):
        xab = np.zeros((128, L), np.float32)
        msel = np.zeros((128, T * 64), np.float32)
        oneh = np.zeros((128, T * 128), np.float32)
        wp = wpack.copy()
        for qq in range(4):
            for i in range(ORDER):
                es = group_lists[(c, qq, i)]
                g = qq * 3 + i
                k = np.arange(len(es))
                x = 128 * g * Tg + k
                t_arr = x // 128
                lane = x % 128
                a, b = (i + 1) % 3, (i + 2) % 3
                ta_t = edge_types[es, a]
                tb_t = edge_types[es, b]
                fa = nodes_b[edges[es, a]].astype(np.float32)  # [m, 13]
                fb = nodes_b[edges[es, b]].astype(np.float32)
                for dd in range(D):
                    xab[13 * ta_t + dd, x] = fa[:, dd]
                    xab[64 + 13 * tb_t + dd, x] = fb[:, dd]
                xab[52 + ta_t, x] = 1.0
                xab[116 + tb_t, x] = 1.0
                p_e = edge_types[es, i]
                for dd in range(D):
                    msel[lane, 64 * t_arr + 4 * dd + p_e] = 1.0
                oneh[lane, 128 * t_arr + r[es, i]] = 1.0
                # bias count matrix: cmat[g=4i+p, q, node_row]
                cnt = np.bincount(r[es, i] + 128 * p_e, minlength=128 * NP_)
                for p in range(NP_):
                    wp[4 * i + p, 448 + 128 * qq:448 + 128 * (qq + 1)] += \
                        cnt[128 * p:128 * (p + 1)]
        import ml_dtypes as _mld
        f8 = _mld.float8_e4m3
        xab_b = xab.astype(bf16)
        oneh_b = oneh.astype(f8)
        msel_b = msel.astype(f8)
        m = {"wpack": wp.astype(bf16)}
        xs = [0, 2, Tg, 2 * Tg, 3 * Tg]  # xab piece boundaries in chunks
        for j in range(4):
            m[f"xab{j}"] = xab_b[:, 512 * xs[j]:512 * xs[j + 1]]
        for j in range(3):
            m[f"msel{j}"] = msel_b[:, j * MT * 64:(j + 1) * MT * 64]
        for j in range(NO):
            m[f"oneh{j}"] = oneh_b[:, j * OT * 128:(j + 1) * OT * 128]
        in_maps.append(m)
    return in_maps, Tg


def kernel(nodes, bp_params, bp_bias, ho_params, ho_bias, edges, edge_types,
           atoms=None, atom_edges=None, _run_kwargs=None):
    from concourse.bass_utils import run_bass_kernel_spmd

    in_maps, Tg = _prep_inputs(nodes, bp_params, bp_bias, ho_params, ho_bias,
                               edges, edge_types)
    if Tg not in _COMPILED:
        _COMPILED[Tg] = _build_program(Tg)
    nc = _COMPILED[Tg]

    res = run_bass_kernel_spmd(nc, in_maps, core_ids=list(range(NCORES)),
                               **(_run_kwargs or {}))
    full = np.zeros((N, D), np.float32)
    for c in range(NCORES):
        oc = res.results[c]["out"]  # [128, 64] = [128, 4q, 16d]
        for qq in range(4):
            full[NODES_PER_CORE * c + 128 * qq:
                 NODES_PER_CORE * c + 128 * (qq + 1), :] = oc[:, 16 * qq:
                                                             16 * qq + 13]
    kernel._last_result = res
    return full
